# revision 1
# baseline (speedup 1.0000x reference)
"""NgramHasher Trainium2 kernel.

Computes h[b,s,ch] = (sum_j coeffs[k,j] * window_j) mod 2^20 for ngram sizes
(2, 3) x 8 tables, on 8 NeuronCores (data parallel over batch).

Math: with c = c0 + 2^10*c1 and t = t0 + 2^10*t1 (all chunks 10-bit),
  h = (A + 2^10 * (B mod 2^10)) mod 2^20
  A = sum_j c0[j]*t0[s-j]          (fp16 matmul, K=10, exact in fp32 PSUM)
  B = sum_j (c0[j]*t1[s-j] + c1[j]*t0[s-j])   (fp16 matmul, K=20)
The "+ 2^10 * u" term rides a scaled-identity (1024*I) fp16 matmul that
accumulates into A's PSUM bank; "mod" ops are DVE tensor_scalar instructions.

Matmul packing: M = 128 = 8 position-offsets (g) x 16 channels; moving columns
are position groups q (position s = 8q + g). Weights are banded Toeplitz.
Device output is [128, 8192] uint32 per core (channel-planar); the host
unshards/permutes to [64, 8192, 16] int64.
"""
import sys
sys.path.insert(0, "/opt/trn_rl_repo")
import numpy as np
from contextlib import ExitStack
from numpy.lib.stride_tricks import sliding_window_view

import concourse.bass as bass
import concourse.tile as tile
from concourse import bacc, mybir
from concourse.bass_utils import run_bass_kernel_spmd

dt = mybir.dt
AluOp = mybir.AluOpType

N_CORES = 8
B, S = 64, 8192
B_LOC = B // N_CORES            # batch rows per core
P_CORE = B_LOC * S              # positions per core (65536)
G = 8                           # position offsets packed into M
NCH = 16                        # output channels (2 ngram sizes x 8 tables)
Q = P_CORE // G                 # moving columns per core (8192)
QCHUNK = 1024                   # columns per pipeline chunk
NCHUNK = Q // QCHUNK

_NC_CACHE = {}


def _build_bass():
    """Build the SPMD Bass program (identical on all 8 cores)."""
    nc = bacc.Bacc("TRN2", target_bir_lowering=False, debug=False,
                   num_devices=N_CORES)
    x_d = nc.dram_tensor("X", [21, Q], dt.float16, kind="ExternalInput").ap()
    wb_d = nc.dram_tensor("WB", [21, 128], dt.float16, kind="ExternalInput").ap()
    wa_d = nc.dram_tensor("WA", [21, 128], dt.float16, kind="ExternalInput").ap()
    wi_d = nc.dram_tensor("WI", [128, 128], dt.float16, kind="ExternalInput").ap()
    out_d = nc.dram_tensor("OUT", [128, Q], dt.uint32, kind="ExternalOutput").ap()

    with tile.TileContext(nc) as tc:
        with ExitStack() as ctx:
            # bufs > NCHUNK: no SBUF slot is ever reused, so DMAs carry no
            # WAR waits (walrus "Too many sync wait commands" otherwise).
            wpool = ctx.enter_context(tc.tile_pool(name="w", bufs=1))
            xpool = ctx.enter_context(tc.tile_pool(name="x", bufs=NCHUNK + 1))
            upool = ctx.enter_context(tc.tile_pool(name="u", bufs=NCHUNK + 1))
            opool = ctx.enter_context(tc.tile_pool(name="o", bufs=NCHUNK + 1))
            psa = ctx.enter_context(tc.tile_pool(name="psa", bufs=2, space="PSUM"))
            psb = ctx.enter_context(tc.tile_pool(name="psb", bufs=2, space="PSUM"))

            w_b = wpool.tile([21, 128], dt.float16, tag="wb")
            nc.sync.dma_start(w_b[:], wb_d[:])
            w_a = wpool.tile([21, 128], dt.float16, tag="wa")
            nc.sync.dma_start(w_a[:], wa_d[:])
            w_i = wpool.tile([128, 128], dt.float16, tag="wi")
            nc.sync.dma_start(w_i[:], wi_d[:])

            # HAM warmup: ~4us of dummy matmuls while the first input DMAs
            # fly, so real matmuls run at 2.4 GHz instead of 1.2.
            wrm = wpool.tile([21, 512], dt.float16, tag="wrm")
            nc.vector.memset(wrm[:], 0.0)

            for ci in range(NCHUNK):
                q0 = ci * QCHUNK
                xt = xpool.tile([21, QCHUNK], dt.float16, tag="xt")
                nc.sync.dma_start(xt[:], x_d[:, q0:q0 + QCHUNK])

                ps_b = psb.tile([128, QCHUNK], dt.float32, tag="psb")
                ps_a = psa.tile([128, QCHUNK], dt.float32, tag="psa")
                if ci == 0:
                    # warmup matmuls overwrite ps_b before its real use
                    for _ in range(10):
                        nc.tensor.matmul(ps_b[:, 0:512], w_b[:], wrm[:],
                                         start=True, stop=True)
                for h in range(QCHUNK // 512):
                    c0, c1 = h * 512, (h + 1) * 512
                    nc.tensor.matmul(ps_b[:, c0:c1], w_b[:],
                                     xt[:, c0:c1], start=True, stop=True)
                for h in range(QCHUNK // 512):
                    c0, c1 = h * 512, (h + 1) * 512
                    nc.tensor.matmul(ps_a[:, c0:c1], w_a[:],
                                     xt[:, c0:c1], start=True, stop=False)

                # PSUM_B holds B + 2^23 (bias row): the fp32 mantissa IS B.
                # Build fp16 bit patterns (exp=25, mantissa=B mod 1024) whose
                # VALUE is 1024 + (B mod 1024), in one DVE op on the low
                # uint16 half of each fp32: (bits & 0x3FF) | (25 << 10).
                u = upool.tile([128, QCHUNK], dt.uint16, tag="u")
                nc.vector.tensor_scalar(
                    u[:], ps_b[:].bitcast(dt.uint16)[:, ::2], 0x3FF, 25 << 10,
                    AluOp.bitwise_and, AluOp.bitwise_or)

                # ps_a += 1024 * (1024 + B mod 1024): the 2^20 bias vanishes
                # mod 2^20; host strips the bias bits with one mask.
                for h in range(QCHUNK // 512):
                    c0, c1 = h * 512, (h + 1) * 512
                    nc.tensor.matmul(ps_a[:, c0:c1], w_i[:],
                                     u[:, c0:c1].bitcast(dt.float16),
                                     start=False, stop=True)

                o = opool.tile([128, QCHUNK], dt.uint32, tag="o")
                nc.scalar.copy(o[:], ps_a[:])
                nc.sync.dma_start(out_d[:, q0:q0 + QCHUNK], o[:])
    nc.compile()
    return nc


def _get_nc():
    if "nc" not in _NC_CACHE:
        _NC_CACHE["nc"] = _build_bass()
    return _NC_CACHE["nc"]


def _band(cpart):
    """[8,3] coeff chunk -> banded Toeplitz [10, 128] weight (fp32 values)."""
    W = np.zeros((10, 128), np.float32)
    for g in range(G):
        for k in range(8):
            for j in range(2):              # ngram n=2 -> channels 0..7
                W[g + 1 + j, g * 16 + k] = cpart[k, j]
            for j in range(3):              # ngram n=3 -> channels 8..15
                W[g + j, g * 16 + 8 + k] = cpart[k, j]
    return W


def _host_prep(token_ids, coeffs):
    t = np.asarray(token_ids).astype(np.int64)
    c = np.asarray(coeffs).astype(np.int64)

    t0 = (t & 0x3FF).astype(np.float16)     # [64, 8192]
    t1 = (t >> 10).astype(np.float16)
    pad = np.zeros((B, 2), np.float16)
    t0p = np.concatenate([pad, t0], axis=1)  # [64, 8194]
    t1p = np.concatenate([pad, t1], axis=1)
    # w?[b, q_loc, r] = t?p[b, 8*q_loc + r],  q_loc in [0,1024), r in [0,10)
    w0 = sliding_window_view(t0p, 10, axis=1)[:, ::G, :]
    w1 = sliding_window_view(t1p, 10, axis=1)[:, ::G, :]
    w0 = np.ascontiguousarray(w0.transpose(0, 2, 1))  # [64, 10, 1024]
    w1 = np.ascontiguousarray(w1.transpose(0, 2, 1))

    c0 = (c & 0x3FF).astype(np.float32)
    c1 = (c >> 10).astype(np.float32)
    # 2^23 bias arrives as (2^15 weight) * (2^8 const input row): both fp16-exact
    bias_row = np.full((1, 128), float(1 << 15), np.float32)
    WB = np.concatenate([_band(c0), _band(c1), bias_row],
                        axis=0).astype(np.float16)
    WA = np.concatenate([np.zeros((10, 128), np.float32), _band(c0), bias_row],
                        axis=0).astype(np.float16)
    WI = (1024.0 * np.eye(128)).astype(np.float16)

    in_maps = []
    for core in range(N_CORES):
        b0 = core * B_LOC
        X = np.empty((21, Q), np.float16)
        # rows 0..9: X1 windows; rows 10..19: X0 windows; row 20: const 1.0
        X[0:10] = w1[b0:b0 + B_LOC].transpose(1, 0, 2).reshape(10, Q)
        X[10:20] = w0[b0:b0 + B_LOC].transpose(1, 0, 2).reshape(10, Q)
        X[20] = 256.0
        in_maps.append({"X": X, "WB": WB, "WA": WA, "WI": WI})
    return in_maps


def _unshard(results):
    out = np.empty((B, S, NCH), np.int64)
    for core, res in enumerate(results):
        o = (res["OUT"] & 0xFFFFF).reshape(G, NCH, Q)  # [g, ch, q]
        o = o.transpose(2, 0, 1).reshape(P_CORE, NCH)  # [8q+g, ch]
        out[core * B_LOC:(core + 1) * B_LOC] = \
            o.reshape(B_LOC, S, NCH).astype(np.int64)
    return out


def _run(token_ids, coeffs, **spmd_kwargs):
    in_maps = _host_prep(token_ids, coeffs)
    nc = _get_nc()
    res = run_bass_kernel_spmd(nc, in_maps, core_ids=list(range(N_CORES)),
                               **spmd_kwargs)
    return _unshard(res.results), res


def kernel(token_ids, coeffs):
    out, _ = _run(token_ids, coeffs)
    return out



# revision 2
# speedup vs baseline: 1.2521x; 1.2521x over previous
"""NgramHasher Trainium2 kernel.

Computes h[b,s,ch] = (sum_j coeffs[k,j] * window_j) mod 2^20 for ngram sizes
(2, 3) x 8 tables, on 8 NeuronCores (data parallel over batch).

Math: with c = c0 + 2^10*c1 and t = t0 + 2^10*t1 (all chunks 10-bit),
  h = (A + 2^10 * (B mod 2^10)) mod 2^20
  A = sum_j c0[j]*t0[s-j]          (fp16 matmul, K=10, exact in fp32 PSUM)
  B = sum_j (c0[j]*t1[s-j] + c1[j]*t0[s-j])   (fp16 matmul, K=20)
The "+ 2^10 * u" term rides a scaled-identity (1024*I) fp16 matmul that
accumulates into A's PSUM bank; "mod" ops are DVE tensor_scalar instructions.

Matmul packing: M = 128 = 8 position-offsets (g) x 16 channels; moving columns
are position groups q (position s = 8q + g). Weights are banded Toeplitz.
Device output is [128, 8192] uint32 per core (channel-planar); the host
unshards/permutes to [64, 8192, 16] int64.

Perf notes (HW-measured):
 - The PE HAM clock gate watches ARRAY ACTIVITY (rows lit), not busy time.
   K=21 matmuls (21/128 rows) never warm the PE past 1.2 GHz; K=128 matmuls
   warm it to 2.4 GHz after ~7us.  So warm up with K=128 dummy matmuls on
   zeroed SBUF while the input DMAs fly.
 - The WI combine matmul for chunk ci is deferred until after chunk ci+1's
   B/A matmuls, so the PE never stalls on the DVE u-extraction (~1.5us).
 - Output DMAs alternate between the two HWDGE rings (sync + scalar).
"""
import sys
sys.path.insert(0, "/opt/trn_rl_repo")
import numpy as np
from contextlib import ExitStack
from numpy.lib.stride_tricks import sliding_window_view

import concourse.bass as bass
import concourse.tile as tile
from concourse import bacc, mybir
from concourse.bass_utils import run_bass_kernel_spmd

dt = mybir.dt
AluOp = mybir.AluOpType

N_CORES = 8
B, S = 64, 8192
B_LOC = B // N_CORES            # batch rows per core
P_CORE = B_LOC * S              # positions per core (65536)
G = 8                           # position offsets packed into M
NCH = 16                        # output channels (2 ngram sizes x 8 tables)
Q = P_CORE // G                 # moving columns per core (8192)
QCHUNK = 1024                   # columns per pipeline chunk
NCHUNK = Q // QCHUNK
N_WARM = 17                     # K=128 warmup matmuls (~7.3us at 1.2 GHz)

_NC_CACHE = {}


def _build_bass():
    """Build the SPMD Bass program (identical on all 8 cores)."""
    nc = bacc.Bacc("TRN2", target_bir_lowering=False, debug=False,
                   num_devices=N_CORES)
    x_d = nc.dram_tensor("X", [21, Q], dt.float16, kind="ExternalInput").ap()
    w2_d = nc.dram_tensor("W2", [21, 256], dt.float16, kind="ExternalInput").ap()
    wi_d = nc.dram_tensor("WI", [128, 128], dt.float16, kind="ExternalInput").ap()
    out_d = nc.dram_tensor("OUT", [128, Q], dt.uint32, kind="ExternalOutput").ap()

    with tile.TileContext(nc) as tc:
        with ExitStack() as ctx:
            # bufs > NCHUNK: no SBUF slot is ever reused, so DMAs carry no
            # WAR waits (walrus "Too many sync wait commands" otherwise).
            wpool = ctx.enter_context(tc.tile_pool(name="w", bufs=1))
            upool = ctx.enter_context(tc.tile_pool(name="u", bufs=NCHUNK + 1))
            opool = ctx.enter_context(tc.tile_pool(name="o", bufs=NCHUNK + 1))
            psa = ctx.enter_context(tc.tile_pool(name="psa", bufs=2, space="PSUM"))
            psb = ctx.enter_context(tc.tile_pool(name="psb", bufs=2, space="PSUM"))

            # HAM warmup scratch: zeroed [128, 640]: cols 0:128 = weights,
            # 128:640 = moving data.  Full-K (128-row) matmuls are what make
            # the HAM clock gate open up to 2.4 GHz.
            wrm = wpool.tile([128, 640], dt.float16, tag="wrm")
            nc.vector.memset(wrm[:], 0.0)

            w_2 = wpool.tile([21, 256], dt.float16, tag="w2")
            nc.sync.dma_start(w_2[:], w2_d[:])
            w_i = wpool.tile([128, 128], dt.float16, tag="wi")
            nc.sync.dma_start(w_i[:], wi_d[:])
            xt = wpool.tile([21, Q], dt.float16, tag="xt")
            nc.sync.dma_start(xt[:], x_d[:])
            w_b = w_2[:, 0:128]
            w_a = w_2[:, 128:256]

            prev = None  # (ps_a, u, ci) awaiting combine+copy+store
            for ci in range(NCHUNK):
                q0 = ci * QCHUNK
                ps_b = psb.tile([128, QCHUNK], dt.float32, tag="psb")
                ps_a = psa.tile([128, QCHUNK], dt.float32, tag="psa")
                if ci == 0:
                    for _ in range(N_WARM):
                        nc.tensor.matmul(ps_b[:, 0:512], wrm[:, 0:128],
                                         wrm[:, 128:640], start=True, stop=True)
                for h in range(QCHUNK // 512):
                    c0, c1 = h * 512, (h + 1) * 512
                    nc.tensor.matmul(ps_b[:, c0:c1], w_b,
                                     xt[:, q0 + c0:q0 + c1],
                                     start=True, stop=True)

                # PSUM_B holds B + 2^23 (bias row): the fp32 mantissa IS B.
                # Build fp16 bit patterns (exp=25, mantissa=B mod 1024) whose
                # VALUE is 1024 + (B mod 1024), in one DVE op on the low
                # uint16 half of each fp32: (bits & 0x3FF) | (25 << 10).
                u = upool.tile([128, QCHUNK], dt.uint16, tag="u")
                nc.vector.tensor_scalar(
                    u[:], ps_b[:].bitcast(dt.uint16)[:, ::2], 0x3FF, 25 << 10,
                    AluOp.bitwise_and, AluOp.bitwise_or)

                for h in range(QCHUNK // 512):
                    c0, c1 = h * 512, (h + 1) * 512
                    nc.tensor.matmul(ps_a[:, c0:c1], w_a,
                                     xt[:, q0 + c0:q0 + c1],
                                     start=True, stop=False)

                if prev is not None:
                    _finish(nc, prev, w_i, opool, out_d)
                prev = (ps_a, u, ci)
            _finish(nc, prev, w_i, opool, out_d)
    nc.compile()
    return nc


def _finish(nc, prev, w_i, opool, out_d):
    """Combine matmul (ps_a += 1024*u), PSUM->SBUF copy, DMA out."""
    ps_a, u, ci = prev
    q0 = ci * QCHUNK
    # ps_a += 1024 * (1024 + B mod 1024): the 2^20 bias vanishes
    # mod 2^20; host strips the bias bits with one mask.
    for h in range(QCHUNK // 512):
        c0, c1 = h * 512, (h + 1) * 512
        nc.tensor.matmul(ps_a[:, c0:c1], w_i[:],
                         u[:, c0:c1].bitcast(dt.float16),
                         start=False, stop=True)
    o = opool.tile([128, QCHUNK], dt.uint32, tag="o")
    nc.scalar.copy(o[:], ps_a[:])
    eng = nc.sync if (ci % 2 == 0) else nc.scalar
    eng.dma_start(out_d[:, q0:q0 + QCHUNK], o[:])


def _get_nc():
    if "nc" not in _NC_CACHE:
        _NC_CACHE["nc"] = _build_bass()
    return _NC_CACHE["nc"]


def _band(cpart):
    """[8,3] coeff chunk -> banded Toeplitz [10, 128] weight (fp32 values)."""
    W = np.zeros((10, 128), np.float32)
    for g in range(G):
        for k in range(8):
            for j in range(2):              # ngram n=2 -> channels 0..7
                W[g + 1 + j, g * 16 + k] = cpart[k, j]
            for j in range(3):              # ngram n=3 -> channels 8..15
                W[g + j, g * 16 + 8 + k] = cpart[k, j]
    return W


def _host_prep(token_ids, coeffs):
    t = np.asarray(token_ids).astype(np.int64)
    c = np.asarray(coeffs).astype(np.int64)

    t0 = (t & 0x3FF).astype(np.float16)     # [64, 8192]
    t1 = (t >> 10).astype(np.float16)
    pad = np.zeros((B, 2), np.float16)
    t0p = np.concatenate([pad, t0], axis=1)  # [64, 8194]
    t1p = np.concatenate([pad, t1], axis=1)
    # w?[b, q_loc, r] = t?p[b, 8*q_loc + r],  q_loc in [0,1024), r in [0,10)
    w0 = sliding_window_view(t0p, 10, axis=1)[:, ::G, :]
    w1 = sliding_window_view(t1p, 10, axis=1)[:, ::G, :]
    w0 = np.ascontiguousarray(w0.transpose(0, 2, 1))  # [64, 10, 1024]
    w1 = np.ascontiguousarray(w1.transpose(0, 2, 1))

    c0 = (c & 0x3FF).astype(np.float32)
    c1 = (c >> 10).astype(np.float32)
    # 2^23 bias arrives as (2^15 weight) * (2^8 const input row): both fp16-exact
    bias_row = np.full((1, 128), float(1 << 15), np.float32)
    WB = np.concatenate([_band(c0), _band(c1), bias_row],
                        axis=0).astype(np.float16)
    WA = np.concatenate([np.zeros((10, 128), np.float32), _band(c0), bias_row],
                        axis=0).astype(np.float16)
    W2 = np.concatenate([WB, WA], axis=1)   # [21, 256]
    WI = (1024.0 * np.eye(128)).astype(np.float16)

    in_maps = []
    for core in range(N_CORES):
        b0 = core * B_LOC
        X = np.empty((21, Q), np.float16)
        # rows 0..9: X1 windows; rows 10..19: X0 windows; row 20: const 1.0
        X[0:10] = w1[b0:b0 + B_LOC].transpose(1, 0, 2).reshape(10, Q)
        X[10:20] = w0[b0:b0 + B_LOC].transpose(1, 0, 2).reshape(10, Q)
        X[20] = 256.0
        in_maps.append({"X": X, "W2": W2, "WI": WI})
    return in_maps


def _unshard(results):
    out = np.empty((B, S, NCH), np.int64)
    for core, res in enumerate(results):
        o = (res["OUT"] & 0xFFFFF).reshape(G, NCH, Q)  # [g, ch, q]
        o = o.transpose(2, 0, 1).reshape(P_CORE, NCH)  # [8q+g, ch]
        out[core * B_LOC:(core + 1) * B_LOC] = \
            o.reshape(B_LOC, S, NCH).astype(np.int64)
    return out


def _run(token_ids, coeffs, **spmd_kwargs):
    in_maps = _host_prep(token_ids, coeffs)
    nc = _get_nc()
    res = run_bass_kernel_spmd(nc, in_maps, core_ids=list(range(N_CORES)),
                               **spmd_kwargs)
    return _unshard(res.results), res


def kernel(token_ids, coeffs):
    out, _ = _run(token_ids, coeffs)
    return out
